# revision 4
# baseline (speedup 1.0000x reference)
"""Winograd F(4,4) causal Conv1d (K=4) + bias + silu for TRN2, 8 NeuronCores.

Reference op: x (B=4, C_IN=2048, S=4096) fp32, weight (C_OUT=2048, C_IN, 4),
bias (C_OUT,);  out = silu(causal_conv1d(x, weight) + bias).

Sharding: data-parallel over sequence as in the direct baseline. Core c
computes out[:, :, c*512:(c+1)*512] from x[:, :, c*512-3 : c*512+512]
(zero-padded left halo), full weight/bias.

Algorithm: Cook-Toom F(4,4) with points [0, 1, -1, 2, -2, 1/2, inf].
Each tile of m=4 output positions needs n=7 Winograd-domain products, so
the PE streams 7/16 of the direct method's contraction columns:
  V[p][ci, bi, t] = sum_q BT[p,q] x[ci, bi, 4t+q-3]      (host, fp32->fp16)
  M[p][co, bi, t] = sum_ci U[p][co, ci] V[p][ci, bi, t]  (PE, fp16 ops, fp32 PSUM)
  y[co, bi, 4t+i] = sum_p AT[i,p] M[p][co, bi, t]        (DVE + ScalarE, fp32)
  out = silu(y + bias)                                   (ScalarE)
U = G @ taps and V = BT @ x-tiles are prepared on the host (input layout
prep, like the baseline's pad/transpose/cast); the conv's contraction —
99.7% of its FLOPs — plus the inverse transform and activation run on
device. Full-size numpy pilot of this exact scheme: rel err 1.73e-3.

Device structure per core: for mi (16) / p (7): one PSUM bank accumulates
16 ki-matmuls of [128,128]x[128,512] fp16 (fp16 gets the compiler's
fast-weight-load, unlike fp32r). Banks rotate (mi*7+p)%8; as each p
completes, DVE folds AT[i,p]*M[p] into fp32 Ysum tiles (ScalarE takes
each phase's first, pure-copy term) while the next p streams — matmuls
never wait on eviction. ScalarE applies silu+bias, writing the
phase-interleaved output tile, and streams it out. V tiles are DMA'd
p-major so the first matmul group's inputs land in ~6us.
"""

import numpy as np

import concourse.bacc as bacc
import concourse.bass as bass
import concourse.mybir as mybir
import concourse.tile as tile
from concourse.bass_utils import run_bass_kernel_spmd

P = 128

B = 4
C_IN = 2048
C_OUT = 2048
KTAPS = 4
S = 4096
N_CORES = 8
S_CHUNK = S // N_CORES          # 512
HALO = KTAPS - 1                # 3

WINO_M = 4                      # outputs per tile
WINO_N = WINO_M + KTAPS - 1     # 7 winograd-domain points
NT = S_CHUNK // WINO_M          # 128 tiles per (batch, core)
N_KI = C_IN // P                # 16
N_MI = C_OUT // P               # 16
POINTS = (0.0, 1.0, -1.0, 2.0, -2.0, 0.5)


def wino_mats():
    """Cook-Toom F(4,4) matrices (float64): AT (4x7), G (7x4), BT (7x7)."""
    m, r = WINO_M, KTAPS
    n = m + r - 1
    a = POINTS
    N = [np.prod([a[i] - a[k] for k in range(n - 1) if k != i])
         for i in range(n - 1)]
    G = np.zeros((n, r))
    for i in range(n - 1):
        G[i] = np.array([a[i] ** j for j in range(r)]) / N[i]
    G[n - 1, r - 1] = 1.0
    BT = np.zeros((n, n))
    for i in range(n - 1):
        poly = np.array([1.0])
        for k in range(n - 1):
            if k == i:
                continue
            poly = np.convolve(poly, np.array([-a[k], 1.0]))
        BT[i, : len(poly)] = poly
    poly = np.array([1.0])
    for k in range(n - 1):
        poly = np.convolve(poly, np.array([-a[k], 1.0]))
    BT[n - 1, : len(poly)] = poly
    AT = np.zeros((m, n))
    for j in range(n - 1):
        AT[:, j] = np.array([a[j] ** i for i in range(m)])
    AT[m - 1, n - 1] = 1.0
    return AT, G, BT


def build_wino_nc(reps=1, act_fn=None):
    AT, _, _ = wino_mats()
    f32 = mybir.dt.float32
    f16 = mybir.dt.float16

    nc = bacc.Bacc("TRN2", target_bir_lowering=False, debug=False)

    v_d = nc.dram_tensor(
        "v", [WINO_N, N_KI, P, B, NT], f16, kind="ExternalInput"
    ).ap()
    w_d = nc.dram_tensor(
        "w", [N_MI, WINO_N, 2, P, 8 * P], f16, kind="ExternalInput"
    ).ap()
    bias_d = nc.dram_tensor(
        "bias", [P, N_MI], f32, kind="ExternalInput"
    ).ap()
    out_d = nc.dram_tensor(
        "out", [B, N_MI * P, S_CHUNK], f32, kind="ExternalOutput"
    ).ap()

    silu = act_fn if act_fn is not None else mybir.ActivationFunctionType.Silu
    copy_fn = mybir.ActivationFunctionType.Copy
    mult = mybir.AluOpType.mult
    add = mybir.AluOpType.add

    ps_banks = [
        nc.alloc_psum_tensor(f"psb{k}", [P, S_CHUNK], f32).ap()
        for k in range(8)
    ]

    # Per-phase AT column structure for the incremental PSUM eviction.
    at_terms = {i: [(p, AT[i, p]) for p in range(WINO_N) if AT[i, p] != 0.0]
                for i in range(WINO_M)}
    first_p = {i: terms[0][0] for i, terms in at_terms.items()}
    at_coef = {(i, p): c for i in range(WINO_M) for p, c in at_terms[i]}

    with tile.TileContext(nc) as tc:
        with (
            tc.tile_pool(name="vpool", bufs=1) as vpool,
            tc.tile_pool(name="wpool", bufs=4) as wpool,
            tc.tile_pool(name="yspool", bufs=2) as yspool,
            tc.tile_pool(name="opool", bufs=2) as opool,
            tc.tile_pool(name="bpool", bufs=1) as bpool,
        ):
            bias_t = bpool.tile([P, N_MI], f32, tag="bias")
            nc.sync.dma_start(out=bias_t, in_=bias_d)

            v_t = {}
            for rep in range(reps):
                # V load, p-major so (mi=0, p=0) inputs arrive first.
                for p in range(WINO_N):
                    for ki in range(N_KI):
                        vt = vpool.tile(
                            [P, B, NT], f16, tag=f"v{ki}_{p}", name="vt"
                        )
                        v_t[ki, p] = vt
                        nc.scalar.dma_start(out=vt, in_=v_d[p, ki])

                for mi in range(N_MI):
                    ys = {}
                    for p in range(WINO_N):
                        ps = ps_banks[(rep * N_MI * WINO_N + mi * WINO_N + p) % 8]
                        for kc in range(2):
                            w_t = wpool.tile([P, 8, P], f16, tag="w",
                                             name="w_t")
                            nc.sync.dma_start(out=w_t, in_=w_d[mi, p, kc])
                            for k8 in range(8):
                                ki = kc * 8 + k8
                                nc.tensor.matmul(
                                    ps,
                                    w_t[:, k8],
                                    v_t[ki, p],
                                    start=(ki == 0),
                                    stop=(ki == N_KI - 1),
                                )
                        for i in range(WINO_M):
                            coef = at_coef.get((i, p))
                            if coef is None:
                                continue
                            if p == first_p[i]:
                                ys_t = yspool.tile(
                                    [P, B, NT], f32, tag=f"ys{i}",
                                    name=f"ys{i}",
                                )
                                ys[i] = ys_t
                                nc.scalar.activation(
                                    ys[i], ps, copy_fn, scale=float(coef)
                                )
                            else:
                                nc.vector.scalar_tensor_tensor(
                                    ys[i], ps, float(coef), ys[i], mult, add
                                )
                    out4 = opool.tile([P, B, NT, WINO_M], f32, tag="o",
                                      name="o4")
                    for i in range(WINO_M):
                        nc.scalar.activation(
                            out4[:, :, :, i], ys[i], silu,
                            bias=bias_t[:, mi : mi + 1],
                        )
                    for bi in range(B):
                        nc.scalar.dma_start(
                            out=out_d[bi, mi * P : (mi + 1) * P, :],
                            in_=out4[:, bi],
                        )
    nc.compile()
    return nc


def prep_inputs(x, weight, bias):
    """Full inputs -> per-core in_maps (host-side layout + transforms)."""
    x = np.asarray(x, dtype=np.float32)
    weight = np.asarray(weight, dtype=np.float32)
    bias = np.asarray(bias, dtype=np.float32)

    AT, G, BT = wino_mats()
    # U[p][co, ci] in float64 -> fp16, laid out [mi, p, kc, part(ci%128), k8, f(co%128)]
    U = np.einsum("pt,oit->poi", G, weight.astype(np.float64))
    U6 = U.reshape(WINO_N, N_MI, P, N_KI, P)          # p, mi, f, ki, part
    w_arr = U6.transpose(1, 0, 3, 4, 2)               # mi, p, ki, part, f
    w_arr = w_arr.reshape(N_MI, WINO_N, 2, 8, P, P)   # mi, p, kc, k8, part, f
    w_arr = w_arr.transpose(0, 1, 2, 4, 3, 5)         # mi, p, kc, part, k8, f
    w_arr = np.ascontiguousarray(
        w_arr.reshape(N_MI, WINO_N, 2, P, 8 * P)
    ).astype(np.float16)

    bias2 = np.ascontiguousarray(bias.reshape(N_MI, P).T)  # (P, n_mi)

    xpad = np.pad(x, ((0, 0), (0, 0), (HALO, 0)))  # (B, C_IN, S+3)
    BT32 = BT.astype(np.float32)
    in_maps = []
    for c in range(N_CORES):
        xc = xpad[:, :, c * S_CHUNK : c * S_CHUNK + S_CHUNK + HALO]  # (B,C_IN,515)
        # d[b, ci, q, t] = xc[b, ci, 4t+q] ; V = BT @ d over q (fp32),
        # stored fp16 as [p, ki, part, b, t].
        d = np.lib.stride_tricks.sliding_window_view(
            xc, WINO_N, axis=2
        )[:, :, ::WINO_M, :]                          # (B, C_IN, NT, n)
        v = np.einsum("pq,bctq->pbct", BT32, d)       # (n, B, C_IN, NT)
        v = v.reshape(WINO_N, B, N_KI, P, NT).transpose(0, 2, 3, 1, 4)
        in_maps.append({
            "v": np.ascontiguousarray(v).astype(np.float16),
            "w": w_arr,
            "bias": bias2,
        })
    return in_maps


def kernel(x, weight, bias):
    in_maps = prep_inputs(x, weight, bias)
    nc = build_wino_nc()

    global LAST_RESULT
    res = run_bass_kernel_spmd(
        nc, in_maps, core_ids=list(range(N_CORES)), trace=PROFILE
    )
    LAST_RESULT = res
    out = np.concatenate([r["out"] for r in res.results], axis=2)
    return out


PROFILE = False
LAST_RESULT = None
